# revision 47
# baseline (speedup 1.0000x reference)
"""Trainium2 Bass kernel for AuxiliaryMultiHeadedAttention.

Reference computation (B=4, S=1024, HID=1024, H=16 heads, DH=64):
    qh  = split_heads(q @ Wq.T + bq)
    kh  = split_heads(k @ Wk.T + bk)
    vh  = split_heads(v @ Wv.T + bv)
    kbh = split_heads(k_b @ Wkb.T + bkb)
    corr = qh @ (kh + kbh).T / sqrt(3*DH)
    corr = where(mask[b, t] == 0, -1e9, corr)          # mask over key positions
    prob = softmax(corr, axis=-1)
    out  = merge_heads(prob @ vh) @ Wo.T + bo

Work decomposition: 8 logical slots = 4 batches x 2 head-groups (8 heads
each).  A slot computes its batch's projections for its 8 heads, attention,
and a partial output projection over its 512 hidden dims.  The host sums the
two partials per batch (replaces the all-reduce) and adds bo.

Slots are mapped onto NDEV physical NeuronCores (KERNEL_NDEV, default 2);
each core runs 8/NDEV slots sequentially inside one NEFF.  The axon tunnel
dispatches per-core executions serially at ~120-155us each, so fewer cores
with more on-core work minimizes per-execution cost; inputs are packed into
4 DRAM tensors (acts / wpack / wopack / consts) because each PJRT buffer
adds ~25-50us of per-execution dispatch overhead.

Device-side layout is feature-major ([feature, token]); the host feeds
pre-transposed activations and weights so no on-chip transposes are needed.
Scores are computed transposed ([t, s]); softmax over t is handled by
multiplying exp tiles against V extended with a mask column on the PE
(the 65th output row of the PV matmul is the softmax denominator), so no
partition-dim reductions are needed.  Matmul inputs are float32r by default
(full PE rate for fp32 data); KERNEL_MM_DT=bf16|f32 selects alternatives.
"""

import math
import os

import numpy as np

import concourse.bass as bass
import concourse.mybir as mybir
import concourse.tile as tile
from concourse import bacc
from concourse.bass_utils import run_bass_kernel_spmd

B, S, HID, H = 4, 1024, 1024, 16
DH = HID // H            # 64
NSLOTS_TOTAL = 8         # 4 batches x 2 head-groups
HPC = H // 2             # 8 heads per slot
DPC = HPC * DH           # 512 hidden dims per slot
P = 128
KT = HID // P            # 8 k-tiles (contraction over hid)
ST = S // P              # 8 s/t-tiles
NB = 512                 # matmul moving free dim (one PSUM bank of fp32)
SC = S // NB             # 2 s-chunks
DT = DPC // P            # 4 d'-tiles
F32 = mybir.dt.float32
SCALE = 1.0 / math.sqrt(3 * DH)

NDEV = int(os.environ.get("KERNEL_NDEV", "8"))
NSLOT = NSLOTS_TOTAL // NDEV         # slots per physical core
NBPC = max(1, NSLOT // 2)            # batches per core
# slot v on core p handles logical index p*NSLOT+v = (batch, head-group)
def _slot_map(p, v):
    idx = p * NSLOT + v
    return idx // 2, idx % 2         # (global batch, head-group)

_MM_NAME = os.environ.get("KERNEL_MM_DT", "bf16")
REPS_IN_NEFF = int(os.environ.get("KERNEL_REPS", "1"))
STAGES = os.environ.get("KERNEL_STAGES", "ABC")
BUFS = {
    "acts": int(os.environ.get("KERNEL_BUFS_ACTS", "20")),
    "wts": int(os.environ.get("KERNEL_BUFS_WTS", "18")),
    "expp": int(os.environ.get("KERNEL_BUFS_EXPP", "10")),
    "ps_sc": int(os.environ.get("KERNEL_BUFS_PSSC", "2")),
    "ps_acc": int(os.environ.get("KERNEL_BUFS_PSACC", "4")),
}
MM_DT = {
    "f32r": mybir.dt.float32r,
    "bf16": mybir.dt.bfloat16,
    "f32": mybir.dt.float32,
}[_MM_NAME]


def _np_mm_dt():
    if _MM_NAME == "bf16":
        import ml_dtypes
        return ml_dtypes.bfloat16
    return np.float32


def _core_groups(p):
    """Distinct head-groups used by core p, in slot order."""
    gs = []
    for v in range(NSLOT):
        _, g = _slot_map(p, v)
        if g not in gs:
            gs.append(g)
    return gs


NG = len(_core_groups(0))            # head-group slices stored per core

PACK1 = os.environ.get("KERNEL_PACK1", "1") == "1"
SZ_WPK = NG * 4 * HID * DPC
SZ_WO = NG * DPC * HID

# Compacted key length: keys with mask==0 contribute nothing (V rows and
# the PV-denominator column are premultiplied by the mask), so the host
# permutes each batch's unmasked keys to the front and the kernel only
# processes the first _SK key positions.  Set from the mask at
# make_in_maps time: 768 when every batch has <=768 unmasked keys
# (binomial(1024,1/2) makes >768 astronomically unlikely), else the full
# 1024 (bit-identical to the uncompacted kernel).  768 keeps both s-chunk
# loops at SC=2 (chunk width 384) and the QK t-tile pairing even.
_SK = None


def _skl():
    return _SK if _SK is not None else S


def build_module(reps=None):
    global REPS_IN_NEFF
    saved_reps = REPS_IN_NEFF
    if reps is not None:
        REPS_IN_NEFF = reps
    nc = bacc.Bacc(
        "TRN2",
        target_bir_lowering=False,
        debug=False,
        num_devices=NDEV,
    )
    skl = _skl()
    sz_q, sz_kv = HID * S, HID * skl
    actsz = sz_q + 3 * sz_kv
    io = {}
    # single big input buffer: per batch [q | k | kb | v] | wpack | wopack
    data = nc.dram_tensor(
        "data", [NBPC * actsz + SZ_WPK + SZ_WO], MM_DT,
        kind="ExternalInput").ap()
    io["qT"], io["kT"], io["kbT"], io["vT"] = [], [], [], []
    for bl in range(NBPC):
        base = bl * actsz
        io["qT"].append(
            data[base:base + sz_q].rearrange("(d s) -> d s", d=HID))
        for i, nm in enumerate(("kT", "kbT", "vT")):
            o = base + sz_q + i * sz_kv
            io[nm].append(
                data[o:o + sz_kv].rearrange("(d s) -> d s", d=HID))
    wbase = NBPC * actsz
    io["wpack"] = data[wbase:wbase + SZ_WPK].rearrange(
        "(g i d m) -> g i d m", g=NG, i=4, d=HID)
    io["wopack"] = data[wbase + SZ_WPK:].rearrange(
        "(g d j) -> g d j", g=NG, d=DPC)
    # consts: [NG,2,DPC] biases (bq, bk+bkb) then [NBPC,skl] float mask
    io["consts"] = nc.dram_tensor(
        "consts", [NG * 2 * DPC + NBPC * skl], F32, kind="ExternalInput").ap()
    io["out"] = nc.dram_tensor(
        "out", [NSLOT, S, HID], F32, kind="ExternalOutput").ap()

    with tile.TileContext(nc) as tc:
        _build_kernel(tc, io)
    nc.compile()
    REPS_IN_NEFF = saved_reps
    return nc


def _build_kernel(tc, io):
    from contextlib import ExitStack

    nc = tc.nc

    with ExitStack() as ctx:
        ctx.enter_context(
            nc.allow_low_precision(reason="matmul inputs intentionally MM_DT")
        )
        singles = ctx.enter_context(tc.tile_pool(name="singles", bufs=1))
        wts = ctx.enter_context(tc.tile_pool(name="wts", bufs=BUFS["wts"]))
        acts = ctx.enter_context(tc.tile_pool(name="acts", bufs=BUFS["acts"]))
        bigacts = ctx.enter_context(tc.tile_pool(
            name="bigacts", bufs=int(os.environ.get("KERNEL_BUFS_BIG", "6"))))
        expp = ctx.enter_context(tc.tile_pool(name="expp", bufs=BUFS["expp"]))
        outp = ctx.enter_context(tc.tile_pool(name="outp", bufs=3))
        smalls = ctx.enter_context(tc.tile_pool(
            name="smalls", bufs=int(os.environ.get("KERNEL_BUFS_SMALLS", "2"))))
        ps_sc = ctx.enter_context(
            tc.tile_pool(name="ps_sc", bufs=BUFS["ps_sc"], space="PSUM"))
        ps_acc = ctx.enter_context(
            tc.tile_pool(name="ps_acc", bufs=BUFS["ps_acc"], space="PSUM"))

        skl = _skl()
        stk = skl // P                       # key t-tiles
        # key-side s-chunks (offset, width, n t-tiles); widths are 128-
        # multiples and may be uneven (skl=640 -> 384+256)
        w0 = ((skl // SC + P - 1) // P) * P
        kchunks = [(0, w0, w0 // P), (w0, skl - w0, (skl - w0) // P)]
        # Resident intermediates, feature-major, reused by every slot.
        QHT = [singles.tile([P, S], MM_DT, tag=f"qht{r}", name=f"qht{r}")
               for r in range(DT)]                            # qh.T   [d', s]
        KSUMT = [singles.tile([P, skl], MM_DT, tag=f"ksumt{r}",
                              name=f"ksumt{r}")
                 for r in range(DT)]                          # (kh+kbh).T
        VHM = [singles.tile([P, HPC, DH + 1], MM_DT, tag=f"vhm{t}",
                            name=f"vhm{t}")
               for t in range(stk)]
        HT = [singles.tile([P, S], MM_DT, tag=f"ht{r}", name=f"ht{r}")
              for r in range(DT)]                             # hidden.T [d', s]

        # Per-group / per-batch constants, loaded once.
        bias_r = io["consts"][0:NG * 2 * DPC].rearrange(
            "(g w t p) -> g w p t", g=NG, w=2, p=P)
        mask_r = io["consts"][NG * 2 * DPC:].rearrange(
            "(b t p) -> b p t", b=NBPC, p=P)
        bq_g, bks_g, mask_b = [], [], []
        for gi in range(NG):
            t = singles.tile([P, DT], F32, tag=f"bq{gi}")
            nc.gpsimd.dma_start(t, bias_r[gi, 0])
            bq_g.append(t)
            t = singles.tile([P, DT], F32, tag=f"bks{gi}")
            nc.gpsimd.dma_start(t, bias_r[gi, 1])
            bks_g.append(t)
        for bl in range(NBPC):
            t = singles.tile([P, stk], F32, tag=f"mask{bl}")
            nc.gpsimd.dma_start(t, mask_r[bl])
            mask_b.append(t)

        q_r = [ap.rearrange("(kt p) s -> p kt s", p=P) for ap in io["qT"]]
        k_r = [ap.rearrange("(kt p) s -> p kt s", p=P) for ap in io["kT"]]
        kb_r = [ap.rearrange("(kt p) s -> p kt s", p=P) for ap in io["kbT"]]
        v_r = [ap.rearrange("(kt p) s -> p kt s", p=P) for ap in io["vT"]]
        wpk_r = io["wpack"].rearrange("g i (kt p) m -> g i p kt m", p=P)
        wo_r = io["wopack"].rearrange("g (it p) j -> g p it j", p=P)

        groups = _core_groups(0)
        pools = dict(singles=singles, wts=wts, acts=acts, bigacts=bigacts,
                     expp=expp, outp=outp, smalls=smalls, ps_sc=ps_sc,
                     ps_acc=ps_acc, QHT=QHT, KSUMT=KSUMT, VHM=VHM, HT=HT)
        for _rep in range(REPS_IN_NEFF):
            for v in range(NSLOT):
                idx = v
                bl = (idx // 2) if NSLOT > 1 else 0
                gi = groups.index(_slot_map(0, v)[1])
                env = dict(
                    pools,
                    qT=q_r[bl], kT=k_r[bl],
                    kbT=kb_r[bl], vT=v_r[bl],
                    wqT=wpk_r[gi, 0], wkT=wpk_r[gi, 1],
                    wkbT=wpk_r[gi, 2], wvT=wpk_r[gi, 3],
                    woT=wo_r[gi],
                    outap=io["out"][v],
                    bq_s=bq_g[gi], bks_s=bks_g[gi], mask_c=mask_b[bl],
                    skl=skl, stk=stk, kchunks=kchunks,
                    slot=f"r{_rep}v{v}",
                )
                _build_body(tc, env)


def _build_body(tc, env):
    nc = tc.nc
    Exp = mybir.ActivationFunctionType.Exp
    wts = env["wts"]; acts = env["acts"]; bigacts = env["bigacts"]
    expp = env["expp"]; outp = env["outp"]; smalls = env["smalls"]
    ps_sc = env["ps_sc"]; ps_acc = env["ps_acc"]
    QHT = env["QHT"]; KSUMT = env["KSUMT"]; VHM = env["VHM"]; HT = env["HT"]
    bq_s = env["bq_s"]; bks_s = env["bks_s"]; mask_c = env["mask_c"]
    skl = env["skl"]; stk = env["stk"]; kchunks = env["kchunks"]
    w0 = kchunks[0][1]
    sv = env["slot"]

    def act_tiles(src, off, w, nm):
        # merged: one wide DMA for all KT k-tiles of this s-chunk
        t = bigacts.tile([P, KT, w], MM_DT, tag="bact",
                         name=f"a_{nm}{sv}_{off}")
        nc.sync.dma_start(t, src[:, :, off:off + w])
        return [t[:, kt, :] for kt in range(KT)]

    def load_w(src, nm):
        # merged: one wide DMA for the whole [HID, DPC] weight slice
        t = bigacts.tile([P, KT, DPC], MM_DT, tag="bact", name=f"w_{nm}{sv}")
        nc.sync.dma_start(t, src)
        return [t[:, kt, :] for kt in range(KT)]

    # ---- Stage A2: KSUMT[d', t] = Wk_g @ k.T + Wkb_g @ k_b.T + bks ----
    # Interleave weight/activation DMAs k-tile-wise so the first matmul
    # can start after ~1MB instead of after all weights.
    # wk/kc first: the PSUM chain consumes all 8 wk tiles before any wkb,
    # so wkb/kbc loads must not delay them.
    wk, wkb, kc0, kbc0 = [], [], [], []
    for kt in range(KT):
        t = wts.tile([P, DPC], MM_DT, tag="w", name=f"w_wkT{sv}_{kt}")
        nc.sync.dma_start(t, env["wkT"][:, kt, :])
        wk.append(t)
        t = acts.tile([P, w0], MM_DT, tag="act", name=f"a_kT0{sv}_{kt}")
        nc.sync.dma_start(t, env["kT"][:, kt, 0:w0])
        kc0.append(t)
    for kt in range(KT):
        t = wts.tile([P, DPC], MM_DT, tag="w", name=f"w_wkbT{sv}_{kt}")
        nc.sync.dma_start(t, env["wkbT"][:, kt, :])
        wkb.append(t)
        t = acts.tile([P, w0], MM_DT, tag="act", name=f"a_kbT0{sv}_{kt}")
        nc.sync.dma_start(t, env["kbT"][:, kt, 0:w0])
        kbc0.append(t)
    for off, w, _ntl in kchunks:
        kc = kc0 if off == 0 else act_tiles(env["kT"], off, w, "kT")
        kbc = kbc0 if off == 0 else act_tiles(env["kbT"], off, w, "kbT")
        for dt_ in range(DT):
            ps = ps_acc.tile([P, w], F32, tag="ps1")
            for kt in range(KT):
                nc.tensor.matmul(
                    ps,
                    lhsT=wk[kt][:, dt_ * P:(dt_ + 1) * P],
                    rhs=kc[kt],
                    start=(kt == 0),
                    stop=False,
                )
            for kt in range(KT):
                nc.tensor.matmul(
                    ps,
                    lhsT=wkb[kt][:, dt_ * P:(dt_ + 1) * P],
                    rhs=kbc[kt],
                    start=False,
                    stop=(kt == KT - 1),
                )
            nc.vector.tensor_scalar_add(
                KSUMT[dt_][:, off:off + w], ps,
                bks_s[:, dt_:dt_ + 1]
            )

    # ---- Stage A1 (c=0): QHT[d', s] = (Wq_g @ q.T) + bq ----
    # Emission order sets DMA priority: Q chunk 0 (feeds the first QK/exp
    # wave), then all of V (the PV chain needs full VHM), then Q chunk 1.
    wq = load_w(env["wqT"], "wqT")

    def a1_chunk(c):
        qc = act_tiles(env["qT"], c * NB, NB, "qT")
        for dt_ in range(DT):
            ps = ps_acc.tile([P, NB], F32, tag="ps1")
            for kt in range(KT):
                nc.tensor.matmul(
                    ps,
                    lhsT=wq[kt][:, dt_ * P:(dt_ + 1) * P],
                    rhs=qc[kt],
                    start=(kt == 0),
                    stop=(kt == KT - 1),
                )
            nc.vector.tensor_scalar_add(
                QHT[dt_][:, c * NB:(c + 1) * NB], ps, bq_s[:, dt_:dt_ + 1]
            )

    a1_chunk(0)

    # ---- Stage A3: VHM[t, h, 0:64] = (v.T_tile.T @ Wv.T + bv) * mask[t];
    #      VHM[t, h, 64] = mask[t] ----
    wv = load_w(env["wvT"], "wvT")
    for off, w, ntl in kchunks:
        vc = act_tiles(env["vT"], off, w, "vT")
        for tl in range(ntl):
            tt = off // P + tl
            ps = ps_acc.tile([P, NB], F32, tag="ps1")
            # bv is separable: sum_t prob*(vh+bv) = PV/denom + bv, and
            # bv flows through the output projection as the constant row
            # bv @ Wo.T, which the host adds at gather time.
            for kt in range(KT):
                nc.tensor.matmul(
                    ps,
                    lhsT=vc[kt][:, tl * P:(tl + 1) * P],
                    rhs=wv[kt],
                    start=(kt == 0),
                    stop=(kt == KT - 1),
                )
            nc.vector.tensor_scalar_mul(
                VHM[tt][:, :, 0:DH],
                ps.rearrange("p (h d) -> p h d", h=HPC),
                mask_c[:, tt:tt + 1],
            )
            nc.vector.tensor_copy(
                VHM[tt][:, :, DH:DH + 1],
                mask_c[:, tt:tt + 1, None].to_broadcast((P, HPC, 1)),
            )

    a1_chunk(1)

    if "B" not in STAGES:
        return
    # ---- Stage B: attention; s-chunk outer (unblocks on half of QHT),
    #      head pairs inner (adjacent QK matmuls hit disjoint PE row
    #      groups: bases 0 and 64) ----
    wot = bigacts.tile([P, DT, S], MM_DT, tag="bact", name=f"w_wo{sv}")
    nc.sync.dma_start(wot, env["woT"])
    wo = {(it, c2): wot[:, it, c2 * NB:(c2 + 1) * NB]
          for it in range(DT) for c2 in range(SC)}
    for c in range(SC):
        for pr in range(HPC // 2):
            r = pr
            # Per-jj-pair exp tiles: each [P, 2, NB] releases after the two
            # PV matmuls that read it, so the next pair's exps can start
            # before this pair's PV finishes.
            npairs = (stk + 1) // 2
            exs = [
                [expp.tile([P, 2, NB], MM_DT, tag="exp",
                           name=f"ex{sv}_{c}_{pr}_{hh}_{jj}")
                 for jj in range(npairs)]
                for hh in range(2)
            ]
            for jj in range(npairs):
                nu = min(2, stk - 2 * jj)   # odd stk: last pair is single
                # High priority: the exp chain is the critical path; let
                # QK matmuls preempt remaining projection matmuls so the
                # ACT engine is fed as early as possible.
                with tc.high_priority():
                    pss = [ps_sc.tile([P, 2, NB], F32, tag="ps2",
                                      name=f"ps2{sv}_{jj}_{i}")
                           for i in range(2)]
                    for u in range(nu):
                        j = jj * 2 + u
                        for hh in range(2):
                            bp = hh * DH
                            nc.tensor.matmul(
                                pss[hh][:, u],
                                lhsT=KSUMT[r][bp:bp + DH, j * P:(j + 1) * P],
                                rhs=QHT[r][bp:bp + DH, c * NB:(c + 1) * NB],
                                start=True,
                                stop=True,
                            )
                    for hh in range(2):
                        nc.scalar.activation(
                            exs[hh][jj][:, 0:nu, :],
                            pss[hh][:, 0:nu, :],
                            mybir.ActivationFunctionType.Exp,
                            bias=0.0, scale=SCALE,
                        )
            for hh in range(2):
                h = 2 * pr + hh
                bp = hh * DH
                # PV with fused denominator (65th row = sum_t exp * mask)
                psh = ps_acc.tile([P, NB], F32, tag="ps1")
                for j in range(stk):
                    nc.tensor.matmul(
                        psh[0:DH + 1, :],
                        lhsT=VHM[j][:, h, :],
                        rhs=exs[hh][j // 2][:, j % 2, :],
                        start=(j == 0),
                        stop=(j == stk - 1),
                    )
                rec = smalls.tile([1, NB], F32, tag="rec")
                nc.vector.reciprocal(rec, psh[DH:DH + 1, :])
                recb = smalls.tile([DH, NB], F32, tag="recb")
                nc.gpsimd.partition_broadcast(recb, rec)
                nc.vector.tensor_mul(
                    HT[r][bp:bp + DH, c * NB:(c + 1) * NB],
                    psh[0:DH, :],
                    recb,
                )

        # ---- Stage C (half): out rows for this s-chunk ----
        if "C" in STAGES:
            for mt in range(c * (ST // SC), (c + 1) * (ST // SC)):
                for c2 in range(SC):
                    ps = ps_acc.tile([P, NB], F32, tag="ps1")
                    for it in range(DT):
                        nc.tensor.matmul(
                            ps,
                            lhsT=HT[it][:, mt * P:(mt + 1) * P],
                            rhs=wo[(it, c2)],
                            start=(it == 0),
                            stop=(it == DT - 1),
                        )
                    ot = outp.tile([P, NB], F32, tag="ot")
                    nc.vector.tensor_copy(ot, ps)
                    nc.sync.dma_start(
                        env["outap"][mt * P:(mt + 1) * P,
                                     c2 * NB:(c2 + 1) * NB], ot
                    )


def make_in_maps(inputs):
    global _SK, _module, _executor, _timing_executors
    inp = {k: np.asarray(v) for k, v in inputs.items()}
    q, k, v, k_b = inp["q"], inp["k"], inp["v"], inp["k_b"]
    mask = inp["mask"]
    f32 = np.float32
    mdt = _np_mm_dt()

    maxc = int(mask.sum(axis=1).max())
    need = next((sk for sk in (640, 768, 896) if maxc <= sk), S)
    if _SK is None:
        _SK = need
    elif need > _SK:
        # a later call's mask doesn't fit the compacted modules: rebuild full
        _SK = S
        _module, _executor = None, None
        _timing_executors = {}
    skl = _skl()
    # stable permutation: unmasked keys first; the tail is real masked keys
    # (mask==0 zeroes their V rows and denominator terms on device)
    perms = [np.argsort(1 - mask[b], kind="stable")[:skl] for b in range(B)]

    in_maps = []
    for core in range(NDEV):
        batches = sorted({_slot_map(core, vv)[0] for vv in range(NSLOT)})
        groups = _core_groups(core)
        blocks = []
        for b in batches:
            pm = perms[b]
            blocks.append(np.ascontiguousarray(q[b].T).astype(mdt).ravel())
            for x in (k, k_b, v):
                blocks.append(
                    np.ascontiguousarray(x[b].T[:, pm]).astype(mdt).ravel())
        wpack = np.stack([
            np.stack([
                np.ascontiguousarray(inp[w][g * DPC:(g + 1) * DPC, :].T)
                for w in ("Wq", "Wk", "Wkb", "Wv")])
            for g in groups
        ]).astype(mdt)                                   # [NG,4,HID,DPC]
        wopack = np.stack([
            np.ascontiguousarray(inp["Wo"][:, g * DPC:(g + 1) * DPC].T)
            for g in groups
        ]).astype(mdt)                                   # [NG,DPC,HID]
        blocks.append(wpack.ravel())
        blocks.append(wopack.ravel())
        consts = np.concatenate(
            [np.stack([
                np.stack([inp["bq"][g * DPC:(g + 1) * DPC],
                          (inp["bk"] + inp["bkb"])[g * DPC:(g + 1) * DPC]])
                for g in groups]).reshape(-1)]
            + [mask[b][perms[b]].astype(f32) for b in batches]
        ).astype(f32)
        in_maps.append({
            "data": np.concatenate(blocks),
            "consts": consts,
        })
    return in_maps


def gather(results, bo, bv_wo):
    out = np.zeros((B, S, HID), np.float32)
    for p in range(NDEV):
        for vv in range(NSLOT):
            b, _g = _slot_map(p, vv)
            out[b] += results[p]["out"][vv]
    out += (np.asarray(bo, dtype=np.float32) + bv_wo[0] + bv_wo[1])
    return out


def bv_wo_terms(inputs):
    bv = np.asarray(inputs["bv"], dtype=np.float64)
    wo = np.asarray(inputs["Wo"], dtype=np.float64)
    return [
        (bv[g * DPC:(g + 1) * DPC] @ wo[:, g * DPC:(g + 1) * DPC].T)
        .astype(np.float32)
        for g in range(2)
    ]


_module = None
_executor = None


def get_module():
    global _module
    if _module is None:
        _module = build_module()
    return _module


class _Executor:
    """Builds the SPMD PJRT executable once; later calls only move data."""

    def __init__(self, nc):
        import jax
        from jax.sharding import Mesh, PartitionSpec, NamedSharding
        from jax.experimental.shard_map import shard_map
        from concourse import bass2jax

        bass2jax.install_neuronx_cc_hook()
        self.jax = jax
        self.nc = nc
        pid = nc.partition_id_tensor.name if nc.partition_id_tensor else None
        in_names, out_names, out_avals, zeros = [], [], [], []
        for alloc in nc.m.functions[0].allocations:
            if not isinstance(alloc, mybir.MemoryLocationSet):
                continue
            name = alloc.memorylocations[0].name
            if alloc.kind == "ExternalInput":
                if name != pid:
                    in_names.append(name)
            elif alloc.kind == "ExternalOutput":
                out_names.append(name)
                shape = tuple(alloc.tensor_shape)
                dtype = mybir.dt.np(alloc.dtype)
                out_avals.append(jax.core.ShapedArray(shape, dtype))
                zeros.append(np.zeros(shape, dtype))
        self.in_names, self.out_names = in_names, out_names
        all_in = in_names + out_names + ([pid] if pid else [])

        def _body(*args):
            operands = list(args)
            if pid:
                operands.append(bass2jax.partition_id_tensor())
            return tuple(bass2jax._bass_exec_p.bind(
                *operands,
                out_avals=tuple(out_avals),
                in_names=tuple(all_in),
                out_names=tuple(out_names),
                lowering_input_output_aliases=(),
                sim_require_finite=True,
                sim_require_nnan=True,
                nc=nc,
            ))

        devices = jax.devices()[:NDEV]
        mesh = Mesh(np.asarray(devices), ("core",))
        spec = PartitionSpec("core")
        self.sharding = NamedSharding(mesh, spec)
        n_args = len(in_names) + len(out_names)
        self.fn = jax.jit(
            shard_map(_body, mesh=mesh, in_specs=(spec,) * n_args,
                      out_specs=(spec,) * len(out_names), check_rep=False),
            keep_unused=True,
        )
        self.zero_dev = [
            jax.device_put(
                np.zeros((NDEV * z.shape[0], *z.shape[1:]), z.dtype),
                self.sharding,
            )
            for z in zeros
        ]
        self.out_shapes = [tuple(a.shape) for a in out_avals]

    def to_device(self, in_maps):
        jax = self.jax
        return [
            jax.device_put(
                np.concatenate(
                    [np.asarray(in_maps[c][n]) for c in range(NDEV)], axis=0
                ),
                self.sharding,
            )
            for n in self.in_names
        ]

    def run(self, in_maps):
        jax = self.jax
        dev_in = self.to_device(in_maps)
        outs = self.fn(*dev_in, *self.zero_dev)
        jax.block_until_ready(outs)
        results = []
        for c in range(NDEV):
            res = {}
            for i, n in enumerate(self.out_names):
                sh = self.out_shapes[i]
                res[n] = np.asarray(outs[i]).reshape(NDEV, *sh)[c]
            results.append(res)
        return results


def get_executor():
    global _executor
    if _executor is None:
        _executor = _Executor(get_module())
    return _executor


_timing_executors = {}


def get_timing_executor(reps):
    """Executor for a NEFF with `reps` sequential body repetitions, used to
    measure steady-state per-iteration hardware time (device work dominates
    the wall measurement, drowning tunnel-latency noise)."""
    if reps not in _timing_executors:
        _timing_executors[reps] = _Executor(build_module(reps=reps))
    return _timing_executors[reps]


def kernel(**inputs):
    global _executor
    in_maps = make_in_maps(inputs)
    last_err = None
    for attempt in range(3):
        try:
            if attempt < 2:
                res = get_executor().run(in_maps)
            else:
                # fall back to the stock runner path
                res = run_bass_kernel_spmd(
                    get_module(), in_maps, core_ids=list(range(NDEV))
                ).results
            return gather(res, inputs["bo"], bv_wo_terms(inputs))
        except Exception as e:  # transient NRT/device errors: rebuild + retry
            last_err = e
            _executor = None
            import time as _time
            _time.sleep(2.0 * (attempt + 1))
    raise last_err
